# revision 8
# baseline (speedup 1.0000x reference)
"""Trainium2 Bass kernel for a causal single-head attention module (v3).

reference computation (per batch b):
    q = x @ Wq; k = x @ Wk; v = x @ Wv          # [s, 128]
    att = softmax(mask(q @ k.T / sqrt(1024)))   # causal
    out = att @ v                               # [s, 128]

Shapes: x [4, 4096, 1024] f32, W* [1024, 128] f32.

Distribution: 8 NeuronCores, 2 per batch.  The 8 sequence blocks (512 rows
each) of a batch are split between its two cores: core 2b owns blocks
{1,3,5,7}, core 2b+1 owns {0,2,4,6}.  This interleaving balances the causal
triangle AND makes the per-core instruction graph identical (SPMD): every
core runs 4 q-tiles whose key-group counts are {2,4,6,8}; the odd core's
extra (non-causal) key group per tile is zeroed via a per-core input scalar.

Each core projects Q for its own rows and K^T/V^T for all 8 blocks
(K/V replicated within the pair; a pair AllGather was tried and lost --
the collective stack costs ~20us of serial latency).  V^T -> natural V
uses the DMA crossbar transpose (off the PE).  W DMAs go FIRST on the
SWDGE queue so the first projection matmul is not gated on mask setup.
xt lands in per-chunk SBUF tiles for precise DMA->PE gating.
Attention runs in the "St" orientation: St[k,q] = Kt_tile.T @ Qt so that
P^T = exp(St) is directly the stationary operand of the AV matmul.
Row sums use DVE partial adds + one ones-vector matmul per key group.
Normalisation and the final [dv, q] -> [q, dv] transpose happen on host
during unshard.
"""

import os
import ml_dtypes
import numpy as np

import concourse.bass as bass
import concourse.bacc as bacc
import concourse.mybir as mybir
import concourse.tile as tile
from concourse.bass_utils import run_bass_kernel_spmd

F32 = mybir.dt.float32
BF16 = mybir.dt.bfloat16

BATCH = 4
SEQ = 4096
EMB = 1024
DK = 128
P = 128
NCORES = 8
SCALE = 1.0 / float(np.sqrt(EMB))

NBLK = 8
HEAVY_BLOCKS = [1, 3, 5, 7]  # core 2b   (exact causal fit)
LIGHT_BLOCKS = [0, 2, 4, 6]  # core 2b+1 (one padded key-group per tile)


def build_nc(seq: int = SEQ):
    blk = seq // NBLK          # 512
    sub = blk // P             # 4 key subtiles per group
    kcols = 4 * blk            # own rows per core (2048)
    xcols = 8 * blk            # own + peer rows (K/V replicated)
    emb_c = EMB // P           # 8 contraction chunks
    nch = kcols // blk         # 4 projection column chunks of 512

    nc = bacc.Bacc("TRN2", target_bir_lowering=False, debug=False,
                   num_devices=NCORES)

    xt = nc.dram_tensor("xt", [EMB, xcols], BF16, kind="ExternalInput")
    wq = nc.dram_tensor("wq", [P, emb_c, DK], BF16, kind="ExternalInput")
    wk = nc.dram_tensor("wk", [P, emb_c, DK], BF16, kind="ExternalInput")
    wv = nc.dram_tensor("wv", [P, emb_c, DK], BF16, kind="ExternalInput")
    pad = nc.dram_tensor("pad", [P, 1], F32, kind="ExternalInput")
    out_o = nc.dram_tensor("out_o", [P, 4 * blk], F32, kind="ExternalOutput")
    out_s = nc.dram_tensor("out_s", [4, blk], F32, kind="ExternalOutput")

    with tile.TileContext(nc) as tc:
        with tc.tile_pool(name="persist", bufs=1) as persist:
            xt_t = [persist.tile([P, xcols], BF16, name=f"xt{c}")
                    for c in range(emb_c)]
            wq_sb = persist.tile([P, emb_c, DK], BF16)
            wk_sb = persist.tile([P, emb_c, DK], BF16)
            wv_sb = persist.tile([P, emb_c, DK], BF16)
            qt_sb = persist.tile([P, 4 * blk], BF16)
            kt_sb = persist.tile([P, 8 * blk], BF16)
            v_sb = persist.tile([P, 8 * sub, P], BF16)
            vt_all = persist.tile([P, xcols], BF16)
            dmask = persist.tile([P, sub, blk], BF16)
            ones_sb = persist.tile([P, 1], BF16)
            pad_sb = persist.tile([P, 1], F32)
            sums_sb = persist.tile([1, 4 * blk], F32)

            # ---- weights FIRST on the SWDGE queue: they gate the PE ----
            for w_dram, w_sb in ((wk, wk_sb), (wv, wv_sb), (wq, wq_sb)):
                nc.gpsimd.dma_start(w_sb[:], w_dram.ap())
            # xt chunks: one full-width DMA per chunk (8KB descriptors);
            # per-chunk tiles give precise DMA -> PE gating; two HWDGE rings.
            for c in range(emb_c):
                eng = nc.sync if c % 2 == 0 else nc.scalar
                eng.dma_start(xt_t[c][:], xt.ap()[c * P:(c + 1) * P, :])
            # constants / masks (after the weight DMAs)
            nc.gpsimd.memset(ones_sb[:], 1.0)
            nc.gpsimd.dma_start(pad_sb[:], pad.ap())
            nc.gpsimd.memset(dmask[:], 1.0)
            for j in range(sub):
                nc.gpsimd.affine_select(
                    out=dmask[:, j, :],
                    in_=dmask[:, j, :],
                    compare_op=mybir.AluOpType.is_ge,
                    fill=0.0,
                    base=-(j * P),
                    pattern=[[1, blk]],
                    channel_multiplier=-1,
                )

            # ---- K^T and V^T projections (both row-halves), chunk-outer
            # so the PE trails the xt DMA chunk arrivals.  V^T -> natural V
            # per 512-block via the DMA crossbar (off the PE).  The Q
            # projection runs between the two halves.
            def kv_half(half):
                lo = half * kcols
                with tc.tile_pool(name=f"kv_psum{half}", bufs=1,
                                  space="PSUM") as kvp:
                    k_ps = [kvp.tile([P, blk], F32, name=f"kps{half}_{n}")
                            for n in range(nch)]
                    v_ps = [kvp.tile([P, blk], F32, name=f"vps{half}_{n}")
                            for n in range(nch)]
                    for c in range(emb_c):
                        for n in range(nch):
                            nc.tensor.matmul(
                                k_ps[n][:], wk_sb[:, c, :],
                                xt_t[c][:, lo + n * blk:lo + (n + 1) * blk],
                                start=(c == 0), stop=(c == emb_c - 1))
                        for n in range(nch):
                            nc.tensor.matmul(
                                v_ps[n][:], wv_sb[:, c, :],
                                xt_t[c][:, lo + n * blk:lo + (n + 1) * blk],
                                start=(c == 0), stop=(c == emb_c - 1))
                    for n in range(nch):
                        nc.vector.tensor_copy(
                            kt_sb[:, lo + n * blk:lo + (n + 1) * blk],
                            k_ps[n][:])
                    for n in range(nch):
                        t = half * nch + n
                        nc.vector.tensor_copy(
                            vt_all[:, t * blk:(t + 1) * blk], v_ps[n][:])
                        eng = nc.sync if n % 2 == 0 else nc.scalar
                        eng.dma_start_transpose(
                            v_sb[:, t * sub:(t + 1) * sub, :],
                            vt_all[:, t * blk:(t + 1) * blk])

            kv_half(0)
            # ---- Q^T projection (own rows only) ----
            with tc.tile_pool(name="q_psum", bufs=2, space="PSUM") as qp:
                for n in range(nch):
                    ps = qp.tile([P, blk], F32, tag="qproj")
                    for c in range(emb_c):
                        nc.tensor.matmul(ps[:], wq_sb[:, c, :],
                                         xt_t[c][:, n * blk:(n + 1) * blk],
                                         start=(c == 0),
                                         stop=(c == emb_c - 1))
                    nc.vector.tensor_copy(qt_sb[:, n * blk:(n + 1) * blk],
                                          ps[:])
            kv_half(1)

            # ---- attention: one pass per q-tile, biggest tile first ----
            halves = 2
            hs = sub // halves
            with (
                tc.tile_pool(name="st_psum", bufs=2, space="PSUM") as stp,
                tc.tile_pool(name="ot_psum", bufs=2, space="PSUM") as otp,
                tc.tile_pool(name="sum_psum", bufs=2, space="PSUM") as smp,
                tc.tile_pool(name="pt_pool", bufs=6) as ptp,
                tc.tile_pool(name="acc_pool", bufs=4) as accp,
                tc.tile_pool(name="ot_sb_pool", bufs=2) as osp,
            ):
                for i in (3, 2, 1, 0):
                    slots = list(range(0, i + 1)) + list(range(4, 5 + i))
                    ot = otp.tile([P, blk], F32, tag="ot", name=f"ot_{i}")
                    sm = smp.tile([1, blk], F32, tag="sm", name=f"sm_{i}")
                    n_mm = 2 * (i + 1) * sub
                    mm = 0
                    qs = qt_sb[:, i * blk:(i + 1) * blk]
                    for si, s in enumerate(slots):
                        pts = []
                        for h in range(halves):
                            st = stp.tile([P, hs * blk], F32, tag="st")
                            for j in range(hs):
                                jj = h * hs + j
                                nc.tensor.matmul(
                                    st[:, j * blk:(j + 1) * blk],
                                    kt_sb[:, s * blk + jj * P:
                                          s * blk + (jj + 1) * P],
                                    qs,
                                    start=True, stop=True)
                            pt = ptp.tile([P, hs * blk], BF16, tag="pt")
                            nc.scalar.activation(
                                pt[:], st[:],
                                mybir.ActivationFunctionType.Exp,
                                bias=0.0, scale=SCALE)
                            if s == i:  # aligned diagonal group
                                nc.vector.tensor_tensor(
                                    pt[:], pt[:],
                                    dmask[:, h * hs:(h + 1) * hs, :]
                                    .rearrange("p s b -> p (s b)"),
                                    mybir.AluOpType.mult)
                            if s == 4 + i:  # pad group (zeroed on light)
                                nc.vector.tensor_scalar_mul(
                                    pt[:], pt[:], pad_sb[:, 0:1])
                            pts.append(pt)
                            # Ot accumulation: Ot[dv, q] += V_tile.T @ Pt
                            for j in range(hs):
                                jj = h * hs + j
                                nc.tensor.matmul(
                                    ot[:],
                                    v_sb[:, s * sub + jj, :],
                                    pt[:, j * blk:(j + 1) * blk],
                                    start=(mm == 0),
                                    stop=(mm == n_mm - 1))
                                mm += 1
                        # row sums: DVE partial adds reduce each slot to one
                        # [P, blk] tile; consecutive slot PAIRS share one
                        # ones-vector matmul (PSUM-accumulated per tile).
                        acc = accp.tile([P, blk], BF16, tag="acc")
                        h0, h1 = pts
                        nc.vector.tensor_tensor(
                            acc[:], h0[:, 0:blk], h0[:, blk:2 * blk],
                            mybir.AluOpType.add)
                        tmp = accp.tile([P, blk], BF16, tag="acc2")
                        nc.vector.tensor_tensor(
                            tmp[:], h1[:, 0:blk], h1[:, blk:2 * blk],
                            mybir.AluOpType.add)
                        nc.vector.tensor_tensor(
                            acc[:], acc[:], tmp[:], mybir.AluOpType.add)
                        if si % 2 == 0:
                            acc_prev = acc
                        else:
                            nc.vector.tensor_tensor(
                                acc[:], acc[:], acc_prev[:],
                                mybir.AluOpType.add)
                            nc.tensor.matmul(sm[:], ones_sb[:, 0:1], acc[:],
                                             start=(si == 1),
                                             stop=(si == len(slots) - 1))
                    nc.vector.tensor_copy(
                        sums_sb[0:1, i * blk:(i + 1) * blk], sm[:])
                    ot_out = osp.tile([P, blk], F32, tag="ot_sb")
                    nc.vector.tensor_copy(ot_out[:], ot[:])
                    nc.sync.dma_start(out_o.ap()[:, i * blk:(i + 1) * blk],
                                      ot_out[:])
                    nc.sync.dma_start(out_s.ap()[i:i + 1, :],
                                      sums_sb[0:1, i * blk:(i + 1) * blk])

    nc.compile()
    return nc


_NC_CACHE = {}


def _get_nc(seq: int):
    if seq not in _NC_CACHE:
        _NC_CACHE[seq] = build_nc(seq)
    return _NC_CACHE[seq]


def make_in_maps(x, Wq, Wk, Wv, seq=None):
    """Host-side sharding: build the 8 per-core input maps."""
    x = np.asarray(x, dtype=np.float32)
    Wq = np.asarray(Wq, dtype=np.float32)
    Wk = np.asarray(Wk, dtype=np.float32)
    Wv = np.asarray(Wv, dtype=np.float32)
    seq = seq or x.shape[1]
    blk = seq // NBLK
    in_maps = []

    def warr(W):
        # [1024, 128] -> [P, emb_chunks, 128] so the device DMA is contiguous
        return np.ascontiguousarray(
            W.reshape(-1, P, DK).transpose(1, 0, 2)).astype(ml_dtypes.bfloat16)

    warrs = {"wq": warr(Wq), "wk": warr(Wk), "wv": warr(Wv)}
    for core in range(NCORES):
        b, h = core // 2, core % 2
        blocks = HEAVY_BLOCKS if h == 0 else LIGHT_BLOCKS
        rows = np.concatenate(
            [np.arange(g * blk, (g + 1) * blk) for g in blocks])
        peer_blocks = LIGHT_BLOCKS if h == 0 else HEAVY_BLOCKS
        rows_peer = np.concatenate(
            [np.arange(g * blk, (g + 1) * blk) for g in peer_blocks])
        all_rows = np.concatenate([rows, rows_peer])
        xt = np.ascontiguousarray(x[b].T[:, all_rows]).astype(
            ml_dtypes.bfloat16)
        padv = np.full((P, 1), 1.0 if h == 0 else 0.0, dtype=np.float32)
        in_maps.append({
            "xt": xt,
            "pad": padv,
            **warrs,
        })
    return in_maps


def unshard(results, seq=None, batch=BATCH):
    seq = seq or SEQ
    blk = seq // NBLK
    out = np.empty((batch, seq, DK), dtype=np.float32)
    for core in range(NCORES):
        b, h = core // 2, core % 2
        blocks = HEAVY_BLOCKS if h == 0 else LIGHT_BLOCKS
        oo = np.asarray(results[core]["out_o"])  # [128, 4*blk]
        ss = np.asarray(results[core]["out_s"])  # [4, blk]
        for i, g in enumerate(blocks):
            o_cols = oo[:, i * blk:(i + 1) * blk]        # [dv, blk]
            out[b, g * blk:(g + 1) * blk, :] = (o_cols / ss[i][None, :]).T
    return out


LAST_EXEC_NS = None
LAST_RESULTS = None


def kernel(x, Wq, Wk, Wv):
    global LAST_EXEC_NS, LAST_RESULTS
    x = np.asarray(x, dtype=np.float32)
    seq = x.shape[1]
    nc = _get_nc(seq)
    in_maps = make_in_maps(x, Wq, Wk, Wv, seq)
    trace = bool(os.environ.get("BASS_KERNEL_TRACE"))
    res = run_bass_kernel_spmd(nc, in_maps, core_ids=list(range(NCORES)),
                               trace=trace)
    LAST_EXEC_NS = res.exec_time_ns
    LAST_RESULTS = res
    return unshard(res.results, seq, x.shape[0])


if __name__ == "__main__":
    rng = np.random.default_rng(0)
    x = rng.standard_normal((BATCH, SEQ, EMB), dtype=np.float32)
    Wq = rng.standard_normal((EMB, DK), dtype=np.float32) / 32
    Wk = rng.standard_normal((EMB, DK), dtype=np.float32) / 32
    Wv = rng.standard_normal((EMB, DK), dtype=np.float32) / 32
    out = kernel(x, Wq, Wk, Wv)
    print("out", out.shape, out.dtype, "exec_ns", LAST_EXEC_NS)
